# revision 2
# baseline (speedup 1.0000x reference)
"""7x7 median blur v2: pair-packed min/max network on fp16, G row-blocks per
instruction.

Measured cost model for this target: every instruction costs ~25-40us nearly
independent of free-dim size or dtype, so wall time ~= instruction count.
v2 therefore minimizes instructions:
- G=4 row blocks batched per instruction (vs 2): fp16 halves SBUF to fit.
- Pair packing: two mutually-ready (hence independent) same-ALU-op plane ops
  fuse into ONE TT instruction via an extra AP dim — each of in0/in1/out
  carries its own arbitrary pair stride, so any ready pair packs.
  238 plane ops -> 120 TT instructions per group.
- All network ops write the full 518-col tile range into 524-wide planes
  (right guard of 6 cols; operand shifts are normalized to da in [0,6]).
  Values outside an op's valid interval are garbage but provably never read
  by valid columns of consumers (checked against the interval analysis).
- fp16 rounding error bound: |median(fp16(x)) - median(x)| <= 2^-11 * 5.5
  ~= 2.7e-3, far under the 2e-2 relative gate.

Totals per core: 3 groups x (4 tap DMAs + 4 fan-in copies + 120 TT + 1 out
DMA) + setup/tail ~= 395 instructions (vs 1448 in the shipped G=2 baseline).
"""

import numpy as np

H = 512
W = 512
C = 3
B = 8
K = 7
PAD = K // 2
WT = W + 2 * PAD     # tile index space width (518)
GUARD_R = 6          # operand shifts da in [0,6] read up to WT-1+6
PW = WT + GUARD_R    # physical plane width (524)
NBLK = H // 128


# ---------------------------------------------------------------- network ---

def _build_dag():
    """Returns (ops, needed, out_id): DCE'd compare-exchange DAG.
    ops[i] = ('in', k) for tap k, or (alu, (a, da), (b, db)), da/db >= 0."""
    ops = []
    cache = {}

    def inp(k):
        key = ("in", k)
        if key not in cache:
            ops.append(key)
            cache[key] = len(ops) - 1
        return (cache[key], 0)

    def mk(op, a, b):
        (ia, da), (ib, db) = a, b
        if (ia, da) > (ib, db):
            (ia, da), (ib, db) = (ib, db), (ia, da)
        base = min(da, db)
        key = (op, ia, da - base, ib, db - base)
        if key not in cache:
            ops.append((op, (ia, da - base), (ib, db - base)))
            cache[key] = len(ops) - 1
        return (cache[key], base)

    def ce(a, b):
        return mk("min", a, b), mk("max", a, b)

    def oe_merge(A, Bl):
        n, m = len(A), len(Bl)
        if n == 0:
            return list(Bl)
        if m == 0:
            return list(A)
        if n == 1 and m == 1:
            lo, hi = ce(A[0], Bl[0])
            return [lo, hi]
        E = oe_merge(A[0::2], Bl[0::2])
        O = oe_merge(A[1::2], Bl[1::2])
        out = [E[0]]
        i = 0
        while i < len(O) and i + 1 < len(E):
            lo, hi = ce(O[i], E[i + 1])
            out += [lo, hi]
            i += 1
        out += O[i:]
        out += E[i + 1:]
        return out

    def shift(ws, dx):
        return [(i, d + dx) for (i, d) in ws]

    def select_rank(A, Bl, r):
        n, m = len(A), len(Bl)
        cands = []
        for i in range(max(0, r - m), min(r, n) + 1):
            j = r - i
            if i == 0:
                cands.append(Bl[j - 1])
            elif j == 0:
                cands.append(A[i - 1])
            else:
                cands.append(mk("max", A[i - 1], Bl[j - 1]))
        while len(cands) > 1:
            nxt = []
            for q in range(0, len(cands) - 1, 2):
                nxt.append(mk("min", cands[q], cands[q + 1]))
            if len(cands) % 2:
                nxt.append(cands[-1])
            cands = nxt
        return cands[0]

    w = [inp(k) for k in range(K)]
    for i, j in [(0, 6), (2, 3), (4, 5), (0, 2), (1, 4), (3, 6), (0, 1),
                 (2, 5), (3, 4), (1, 2), (4, 6), (2, 3), (4, 5), (1, 2),
                 (3, 4), (5, 6)]:
        lo, hi = ce(w[i], w[j])
        w[i], w[j] = lo, hi
    col = w
    m2 = oe_merge(col, shift(col, 1))
    m3 = oe_merge(m2, shift(col, 2))
    n42 = oe_merge(shift(m3, -3), m3)
    out_id, out_dx = select_rank(n42, shift(col, 3), 25)
    assert out_dx == -3

    # DCE
    needed = set()
    stack = [out_id]
    while stack:
        i = stack.pop()
        if i in needed:
            continue
        needed.add(i)
        op = ops[i]
        if op[0] != "in":
            stack.append(op[1][0])
            stack.append(op[2][0])

    # full-width-evaluation soundness: every op's needed interval lies
    # within its valid interval, so columns that matter only ever read
    # columns that are themselves valid.
    sched = [i for i in range(len(ops)) if i in needed]
    valid = {}
    for i in sched:
        op = ops[i]
        if op[0] == "in":
            valid[i] = (0, WT - 1)
        else:
            _, (a, da), (b, db) = op
            lo = max(valid[a][0] - da, valid[b][0] - db, 0)
            hi = min(valid[a][1] - da, valid[b][1] - db, WT - 1)
            valid[i] = (lo, hi)
    need_iv = {out_id: (0, W - 1)}
    for i in reversed(sched):
        op = ops[i]
        if op[0] == "in" or i not in need_iv:
            continue
        lo, hi = need_iv[i]
        for (a, da) in (op[1], op[2]):
            nlo, nhi = lo + da, hi + da
            if a in need_iv:
                nlo = min(nlo, need_iv[a][0])
                nhi = max(nhi, need_iv[a][1])
            need_iv[a] = (nlo, nhi)
    for i in sched:
        if ops[i][0] != "in":
            assert need_iv[i][0] >= valid[i][0] and need_iv[i][1] <= valid[i][1]

    return ops, needed, out_id


def _pack_schedule():
    """Greedy pair-packing schedule with slot allocation.

    Returns (instrs, n_slots): instrs = list of (alu, lanes), lanes = 1 or 2
    of (out_slot, (a_slot, da), (b_slot, db)); slots < 0 encode taps
    (slot -1-k = tap k); the final op is last, alone, out_slot='final'.
    """
    ops, needed, out_id = _build_dag()
    comp = [i for i in range(len(ops)) if i in needed and ops[i][0] != "in"]

    users = {}
    for i in comp:
        for a in {ops[i][1][0], ops[i][2][0]}:
            users.setdefault(a, set()).add(i)
    users.setdefault(out_id, set()).add(-1)

    indeg = {}
    succs = {i: [] for i in comp}
    for i in comp:
        d = 0
        for a in {ops[i][1][0], ops[i][2][0]}:
            if ops[a][0] != "in":
                d += 1
                succs[a].append(i)
        indeg[i] = d

    height = {}
    for i in sorted(comp, reverse=True):
        h = 0
        for s in succs[i]:
            h = max(h, height.get(s, 0) + 1)
        height[i] = h

    slot_of = {}
    for i in range(len(ops)):
        if i in needed and ops[i][0] == "in":
            slot_of[i] = -1 - ops[i][1]

    free = []
    n_slots = 0
    remaining = {a: set(u) for a, u in users.items()}

    def alloc():
        nonlocal n_slots
        if free:
            return free.pop()
        n_slots += 1
        return n_slots - 1

    def retire(i):
        for a in {ops[i][1][0], ops[i][2][0]}:
            remaining[a].discard(i)
            if not remaining[a] and slot_of.get(a, 0) >= 0 and a != out_id:
                free.append(slot_of[a])

    ready = [i for i in comp if indeg[i] == 0]
    instrs = []
    scheduled = set()

    def frees(i):
        f = 0
        for a in {ops[i][1][0], ops[i][2][0]}:
            if remaining.get(a) == {i} and slot_of.get(a, -1) >= 0:
                f += 1
        return f

    def opcls(a):
        return 0 if ops[a][0] == "in" else 1  # tap vs slot

    def sig(i):
        """(alu, sorted operand tensor-class multiset) — pack-compat key.
        min/max are commutative so operand order can be swapped per lane."""
        _, (a, _), (b, _) = ops[i]
        return (ops[i][0], tuple(sorted((opcls(a), opcls(b)))))

    def emit(lane_ids):
        lanes = []
        for i in lane_ids:
            s = alloc() if i != out_id else "final"
            slot_of[i] = s
            o, (a, da), (b, db) = ops[i]
            # canonical operand order: tap-class first (aligns classes
            # positionally across pack lanes; min/max commute)
            if opcls(a) > opcls(b):
                (a, da), (b, db) = (b, db), (a, da)
            lanes.append((s, (slot_of[a], da), (slot_of[b], db)))
        for i in lane_ids:
            retire(i)
        instrs.append((ops[lane_ids[0]][0], lanes))
        for i in lane_ids:
            scheduled.add(i)
            for s2 in succs[i]:
                indeg[s2] -= 1
                if indeg[s2] == 0:
                    ready.append(s2)

    while ready:
        ready.sort(key=lambda i: (-frees(i), -height[i]))
        cands = [i for i in ready if i != out_id]
        if not cands:
            assert ready == [out_id]
            emit([ready.pop()])
            continue
        first = cands[0]
        mate = None
        for j in cands[1:]:
            if sig(j) == sig(first):
                mate = j
                break
        if mate is not None:
            ready.remove(first)
            ready.remove(mate)
            emit([first, mate])
        else:
            ready.remove(first)
            emit([first])

    assert len(scheduled) == len(comp)
    return instrs, n_slots


# ----------------------------------------------------------------- kernel ---

_CACHE = {}


def _reduce_waits(nc, mybir):
    """Transitive reduction of semaphore waits (same algorithm as baseline)."""
    import bisect
    from collections import defaultdict

    f = nc.m.functions[0]
    insts = [ins for blk in f.blocks for ins in blk.instructions]

    cum = defaultdict(int)
    sem_hist = defaultdict(lambda: ([], []))
    bad_sems = set()
    for idx, ins in enumerate(insts):
        si = ins.sync_info
        if not si:
            continue
        for up in (si.on_update or []):
            if getattr(up, "update_mode", None) in ("sem-inc", "sem-add-imm"):
                cum[up.id] += up.update_value
                vals, idxs = sem_hist[up.id]
                vals.append(cum[up.id])
                idxs.append(idx)
            else:
                bad_sems.add(up.id)

    def achiever(sem, v):
        if sem in bad_sems:
            return None
        vals, idxs = sem_hist.get(sem, ([], []))
        i = bisect.bisect_left(vals, v)
        return idxs[i] if i < len(vals) else None

    know = [None] * len(insts)
    last_on_proc = {}
    reducible = ("InstDMACopy", "InstTensorTensor", "InstTensorCopy",
                 "InstMemset")
    still_multi = []
    for idx, ins in enumerate(insts):
        proc = getattr(ins, "bass_scheduled_proc", None)
        base = {}
        if proc is not None and proc in last_on_proc:
            base = dict(know[last_on_proc[proc]])
        si = ins.sync_info
        waits = list(si.on_wait or []) if si else []
        usable = [w for w in waits
                  if getattr(w, "wait_mode", None) == "sem-ge-imm"
                  and w.wait_reg is None and w.id not in bad_sems]
        cur = dict(base)
        kept = list(waits)
        if si and len(waits) > 1 and len(usable) == len(waits):
            wk = []
            for w in waits:
                a = achiever(w.id, w.wait_value)
                k = dict(know[a]) if (a is not None and know[a]) else {}
                k[w.id] = max(k.get(w.id, 0), w.wait_value)
                wk.append(k)
            order = sorted(range(len(waits)), key=lambda i: -len(wk[i]))
            keep_idx = []
            for wi in order:
                w = waits[wi]
                if cur.get(w.id, 0) >= w.wait_value:
                    continue
                keep_idx.append(wi)
                for s, v in wk[wi].items():
                    cur[s] = max(cur.get(s, 0), v)
            kept = [waits[i] for i in sorted(keep_idx)]
            if len(kept) < len(waits):
                ins.sync_info = mybir.SyncInfo(
                    on_wait=kept, on_update=list(si.on_update or []))
        else:
            for w in usable:
                a = achiever(w.id, w.wait_value)
                if a is not None and know[a]:
                    for s, v in know[a].items():
                        cur[s] = max(cur.get(s, 0), v)
                cur[w.id] = max(cur.get(w.id, 0), w.wait_value)
        if len(kept) > 1 and ins.__class__.__name__ in reducible:
            still_multi.append((ins.name, ins.__class__.__name__,
                                [(w.ant_name, w.wait_value) for w in kept]))
        if si:
            for up in (si.on_update or []):
                if getattr(up, "update_mode", None) in ("sem-inc", "sem-add-imm"):
                    vals, idxs = sem_hist[up.id]
                    i = bisect.bisect_left(idxs, idx)
                    if i < len(idxs) and idxs[i] == idx:
                        cur[up.id] = max(cur.get(up.id, 0), vals[i])
        know[idx] = cur
        if proc is not None:
            last_on_proc[proc] = idx
    return still_multi


def _get_bass(repeat=1, G=4):
    key = ("nc2", repeat, G)
    if key in _CACHE:
        return _CACHE[key]
    import sys
    for p in ("/opt/trn_rl_repo", "/root/.axon_site/_ro/trn_rl_repo"):
        if p not in sys.path:
            sys.path.append(p)
    import concourse.bass as bass
    import concourse.tile as tile
    from concourse import mybir

    instrs, n_slots = _pack_schedule()
    n_units = C * NBLK
    assert n_units % G == 0

    fp16 = mybir.dt.float16
    nc = bass.Bass("TRN2", target_bir_lowering=False, debug=False)
    # column-padded staging: rows of width PW with the 3-col zero pads and
    # right guard baked in by the host, so tap DMAs are one fat contiguous
    # line per partition instead of 7 short strided lines
    img_pad = nc.dram_tensor("img_pad", [C, H + 2 * PAD, PW],
                             fp16, kind="ExternalInput").ap()
    # fat output staging: [C, 128, G*WT]; host slices back to [C, H, W]
    out = nc.dram_tensor("out", [C, 128, NBLK * WT], fp16,
                         kind="ExternalOutput").ap()

    with tile.TileContext(nc) as tc:
        with tc.tile_pool(name="taps", bufs=1) as tap_pool, \
             tc.tile_pool(name="aux", bufs=1) as aux_pool, \
             tc.tile_pool(name="slots", bufs=1) as slot_pool:
            # layout note: slot/tap index is the SECOND-minor dim so that
            # pack-pair strides (slot deltas x PW) stay inside the 16-bit
            # ISA step field; G is an outer AP dim (stride n_slots*PW).
            big = slot_pool.tile([128, G, n_slots, PW], fp16, tag="big",
                                 name="big")
            nc.vector.memset(big[:, :, 0:n_slots // 2, :], 0.0)
            nc.vector.memset(big[:, :, n_slots // 2:, :], 0.0)
            # single-buffered taps (instruction-count metric; overlap
            # across groups is irrelevant, and this keeps SBUF under budget)
            taps = tap_pool.tile([128, G, K, PW], fp16, tag="taps",
                                 name="taps")
            nc.vector.memset(taps[:, :, :, :], 0.0)
            ft_b = []
            for bi in range(2):
                t = aux_pool.tile([128, G, WT], fp16, tag=f"ft{bi}",
                                  name=f"ft{bi}")
                nc.vector.memset(t[:, :, :], 0.0)
                ft_b.append(t)
            scr_b = aux_pool.tile([128, 1], fp16, tag="scr_b", name="scr_b")
            nc.vector.memset(scr_b[:, :], 0.0)

            groups = list(range(0, n_units, G)) * repeat

            pstride_big = n_slots * G * PW
            pstride_tap = K * G * PW
            pstride_ft = G * WT
            gstride_big = n_slots * PW
            gstride_tap = K * PW

            def dma_taps(gi):
                # ONE fat DMA per group: for partition p, chunk g, the 7 tap
                # rows are K*PW contiguous elements both in the staged DRAM
                # (consecutive padded rows) and in SBUF (g-outer layout)
                u0 = groups[gi]
                c, b0 = u0 // NBLK, u0 % NBLK
                base = img_pad[c, 128 * b0:128 * b0 + 1, 0:PW]
                srcap = bass.AP(tensor=base.tensor, offset=base.offset,
                                ap=[[PW, 128], [128 * PW, G], [1, K * PW]])
                t0 = taps[:, 0, 0, 0:1]
                dstap = bass.AP(tensor=t0.tensor, offset=t0.offset,
                                ap=[[pstride_tap, 128], [gstride_tap, G],
                                    [1, K * PW]])
                nc.sync.dma_start(out=dstap, in_=srcap)

            for gi, u0 in enumerate(groups):
                dma_taps(gi)
                ft = ft_b[gi % 2]

                # fan-in: one copy absorbs the single tap-DMA queue wait
                nc.vector.tensor_copy(scr_b[:, :],
                                      taps[:, G - 1, 0, PAD:PAD + 1])
                tc.no_sync_barrier()

                def addr(s, da):
                    """(tensor, gstride, element offset) for one lane."""
                    if s == "final":
                        return (ft, WT, da)
                    if s < 0:
                        k = -1 - s
                        return (taps, gstride_tap, k * PW + da)
                    return (big, gstride_big, s * PW + da)

                pstride = {id(ft): pstride_ft, id(taps): pstride_tap,
                           id(big): pstride_big}

                for (alu, lanes) in instrs:
                    npack = len(lanes)
                    op = (mybir.AluOpType.min if alu == "min"
                          else mybir.AluOpType.max)
                    aps = []
                    for operand in range(3):  # out, in0, in1
                        if operand == 0:
                            parts = [(l[0], 0) for l in lanes]
                        else:
                            parts = [l[operand] for l in lanes]
                        t0, gs0, off0 = addr(*parts[0])
                        dims = [[pstride[id(t0)], 128]]
                        if npack > 1:
                            t1, gs1, off1 = addr(*parts[1])
                            assert t1 is t0, "pack lanes must share tensor"
                            delta = off1 - off0
                            assert abs(delta) < 32768, delta
                            dims.append([delta, 2])
                        dims.append([gs0, G])
                        dims.append([1, WT])
                        tb = (t0[:, 0, 0:1] if t0 is ft
                              else t0[:, 0, 0, 0:1])
                        aps.append(bass.AP(tensor=tb.tensor,
                                           offset=tb.offset + off0, ap=dims))
                    nc.vector.tensor_tensor(out=aps[0], in0=aps[1],
                                            in1=aps[2], op=op)

                # fat out DMA: ft's [G, WT] span is contiguous per partition
                # and lands contiguously in the output staging row
                u0c, u0b = u0 // NBLK, u0 % NBLK
                assert u0b + G <= NBLK
                ob = out[u0c, 0:1, u0b * WT:(u0b + G) * WT]
                odst = bass.AP(tensor=ob.tensor, offset=ob.offset,
                               ap=[[NBLK * WT, 128], [1, G * WT]])
                f0 = ft[:, 0, 0:1]
                fsrc = bass.AP(tensor=f0.tensor, offset=f0.offset,
                               ap=[[pstride_ft, 128], [1, G * WT]])
                nc.sync.dma_start(out=odst, in_=fsrc)

            # tail ladder: DVE observes the last two out-DMAs so the
            # framework drain needs only a single DVE wait
            for ftl in (ft_b[(len(groups) - 1) % 2], ft_b[len(groups) % 2]):
                nc.vector.memset(ftl[:, 0, 0:1], 0.0)
                tc.no_sync_barrier()

    leftover = _reduce_waits(nc, mybir)
    assert not leftover, f"multi-wait instructions remain: {leftover[:5]}"
    _CACHE[key] = nc
    return nc


def kernel(img: np.ndarray) -> np.ndarray:
    import sys
    for p in ("/opt/trn_rl_repo", "/root/.axon_site/_ro/trn_rl_repo"):
        if p not in sys.path:
            sys.path.append(p)
    from concourse.bass_utils import run_bass_kernel_spmd

    img = np.asarray(img, dtype=np.float32)
    assert img.shape == (B, C, H, W), img.shape
    nc = _get_bass(G=4)
    # staging: rows padded to PW columns (3 zero cols each side + guard),
    # 3 zero rows top/bottom
    stage = np.zeros((B, C, H + 2 * PAD, PW), np.float16)
    stage[:, :, PAD:PAD + H, PAD:PAD + W] = img.astype(np.float16)
    in_maps = [{"img_pad": np.ascontiguousarray(stage[b])} for b in range(B)]
    res = run_bass_kernel_spmd(nc, in_maps, list(range(B)))
    # unstage: out[c, p, b*WT + x] -> [c, 128*b + p, x]
    full = np.empty((B, C, H, W), np.float32)
    for b in range(B):
        o = res.results[b]["out"].reshape(C, 128, NBLK, WT)
        full[b] = o[:, :, :, 0:W].transpose(0, 2, 1, 3).reshape(C, H, W)
    return full
